# revision 7
# baseline (speedup 1.0000x reference)
# Trainium2 Bass kernel for NonLocalBlock (B=4, C=64, CI=32, H=W=80).
#
# Math (per batch, N = H*W = 6400):
#   u = Wu@x+bu, v = Wv@x+bv, g = Wg@x+bg           [CI, N]
#   f[n,m] = sum_c u[c,n] v[c,m]; softmax over n (axis=1 of f)
#   y[c,n] = sum_m f_sm[n,m] g[c,m];  out = Ww@y + bw + x
#
# Define S = v^T u  (S[m,n] = f[n,m]).  The softmax axis n is then the
# FREE axis of S rows, so processing S in 128-row blocks makes the
# softmax fully row-local.  y = g @ softmax_rows(S).
#
# Sharding: 8 cores = 4 batches x 2 halves of the m axis.  Each core
# computes a partial y (sum over its 3200 m rows), applies the output
# projection, and the host adds the two halves (bias+residual are
# carried by the odd core via the `resid` input; even cores get zeros).
#
# Numerics: softmax computed WITHOUT max-subtraction: |S| <~ 40 here
# (inputs are unit-normal-ish), exp stays comfortably inside f32 range,
# and exp(S)/sum(exp(S)) is mathematically identical to the reference.
# Row sums come for free from the activation's accum_out; the 1/rowsum
# is folded into the [128,32] g^T block (lhsT of the y matmul) instead
# of rescaling the [128,6400] exp(S) tile.

import numpy as np

import concourse.bass as bass
import concourse.mybir as mybir
from concourse import bacc, tile
from concourse.bass_utils import run_bass_kernel_spmd

F32 = mybir.dt.float32

B, C, CI, H, W = 4, 64, 32, 80, 80
N = H * W              # 6400
NCORES = 8
MH = N // 2            # 3200 rows of S per core
MB = 128               # S row-block
NBLK = MH // MB        # 25 blocks per core
SCH = 1024             # S free-dim chunk held in PSUM (2 banks)
YCH = 512              # y free-dim chunk (1 bank)

EXP = mybir.ActivationFunctionType.Exp


def _ceil_chunks(total, step):
    out = []
    off = 0
    while off < total:
        out.append((off, min(step, total - off)))
        off += step
    return out


S_CHUNKS = _ceil_chunks(N, SCH)      # 6 x 1024 + 256
Y_CHUNKS = _ceil_chunks(N, YCH)      # 12 x 512 + 256
U_CHUNKS = _ceil_chunks(N, 512)      # projection chunks
V_CHUNKS = _ceil_chunks(MH, 512)


def build_nc():
    nc = bacc.Bacc("TRN2", target_bir_lowering=False, debug=False,
                   num_devices=NCORES)

    x_aug_d = nc.dram_tensor("x_aug", [C + 1, N], F32, kind="ExternalInput")
    x_m_d = nc.dram_tensor("x_m", [C + 1, MH], F32, kind="ExternalInput")
    wuT_d = nc.dram_tensor("wuT", [C + 1, CI], F32, kind="ExternalInput")
    wvT_d = nc.dram_tensor("wvT", [C + 1, CI], F32, kind="ExternalInput")
    wgT_d = nc.dram_tensor("wgT", [C + 1, CI], F32, kind="ExternalInput")
    wwT4_d = nc.dram_tensor("wwT4", [128, C], F32, kind="ExternalInput")
    resid_d = nc.dram_tensor("resid", [C, N], F32, kind="ExternalInput")
    out_d = nc.dram_tensor("out", [C, N], F32, kind="ExternalOutput")

    with tile.TileContext(nc) as tc:
        with (
            tc.tile_pool(name="const", bufs=1) as cpool,
            tc.tile_pool(name="big", bufs=2) as dpool,
            tc.tile_pool(name="small", bufs=3) as wpool,
            tc.tile_pool(name="ypsum", bufs=1, space="PSUM") as ypool,
        ):
            # ---- persistent SBUF tiles ----
            x_aug = cpool.tile([C + 1, N], F32, tag="xa")
            x_m = cpool.tile([C + 1, MH], F32, tag="xm")
            u_sb = cpool.tile([CI, N], F32, tag="u")
            v_sb = cpool.tile([CI, MH], F32, tag="v")
            gt_sb = cpool.tile([128, NBLK * CI], F32, tag="gt")
            wuT = cpool.tile([C + 1, CI], F32, tag="wu")
            wvT = cpool.tile([C + 1, CI], F32, tag="wv")
            wgT = cpool.tile([C + 1, CI], F32, tag="wg")
            wwT4 = cpool.tile([128, C], F32, tag="ww")
            resid = cpool.tile([C, N], F32, tag="resid")
            y_sb = cpool.tile([128, 4 * YCH], F32, tag="ysb")

            # ---- input DMAs (chunked for queue parallelism) ----
            for k in range(4):
                s = slice(k * (N // 4), (k + 1) * (N // 4))
                nc.sync.dma_start(x_aug[:, s], x_aug_d[:, s])
            for k in range(2):
                s = slice(k * (MH // 2), (k + 1) * (MH // 2))
                nc.sync.dma_start(x_m[:, s], x_m_d[:, s])
            nc.sync.dma_start(wuT[:], wuT_d[:])
            nc.sync.dma_start(wvT[:], wvT_d[:])
            nc.sync.dma_start(wgT[:], wgT_d[:])
            nc.sync.dma_start(wwT4[:], wwT4_d[:])
            for k in range(4):
                s = slice(k * (N // 4), (k + 1) * (N // 4))
                nc.sync.dma_start(resid[:, s], resid_d[:, s])

            # ---- projections: u (full), v (this core's m range), g^T ----
            with tc.tile_pool(name="ppsum", bufs=2, space="PSUM") as ppool:
                for off, cw in U_CHUNKS:
                    pu = ppool.tile([CI, 512], F32, tag="pu")
                    nc.tensor.matmul(pu[:, :cw], wuT[:], x_aug[:, off:off + cw],
                                     start=True, stop=True)
                    nc.scalar.copy(u_sb[:, off:off + cw], pu[:, :cw])
                for off, cw in V_CHUNKS:
                    pv = ppool.tile([CI, 512], F32, tag="pu")
                    nc.tensor.matmul(pv[:, :cw], wvT[:], x_m[:, off:off + cw],
                                     start=True, stop=True)
                    nc.vector.tensor_copy(v_sb[:, off:off + cw], pv[:, :cw])
                for i in range(NBLK):
                    pg = ppool.tile([128, CI], F32, tag="pg")
                    nc.tensor.matmul(pg[:], x_m[:, i * MB:(i + 1) * MB], wgT[:],
                                     start=True, stop=True)
                    nc.vector.tensor_copy(gt_sb[:, i * CI:(i + 1) * CI], pg[:])

            # ---- y accumulators: 13 chunks packed 4-per-bank ----
            y_ps = [ypool.tile([128, YCH], F32, tag=f"y{t}", name=f"y{t}")
                    for t in range(4)]

            def y_slot(j):
                return y_ps[j // 4][32 * (j % 4):32 * (j % 4) + 32, :]

            gts_prev = None
            exp_prev = None

            with tc.tile_pool(name="spsum", bufs=2, space="PSUM") as spool:
                for i in range(NBLK):
                    vblk = v_sb[:, i * MB:(i + 1) * MB]
                    exp_t = dpool.tile([128, N], F32, tag="expS")
                    sums = wpool.tile([128, len(S_CHUNKS)], F32, tag="sums")

                    for ci, (off, cw) in enumerate(S_CHUNKS):
                        sp = spool.tile([128, SCH], F32, tag="s")
                        for s2 in range(0, cw, 512):
                            w2 = min(512, cw - s2)
                            nc.tensor.matmul(
                                sp[:, s2:s2 + w2], vblk,
                                u_sb[:, off + s2:off + s2 + w2],
                                start=True, stop=True)
                        nc.scalar.activation(
                            exp_t[:, off:off + cw], sp[:, :cw], EXP,
                            accum_out=sums[:, ci:ci + 1])

                    # y matmuls for the previous block (keeps ACT busy:
                    # this block's S matmuls were already emitted above)
                    if i > 0:
                        for j, (off, cw) in enumerate(Y_CHUNKS):
                            nc.tensor.matmul(
                                y_slot(j)[:, :cw], gts_prev,
                                exp_prev[:, off:off + cw],
                                start=(i - 1 == 0), stop=(i - 1 == NBLK - 1),
                                tile_position=(0, 32 * (j % 4)),
                                skip_group_check=True)

                    tot = wpool.tile([128, 1], F32, tag="tot")
                    nc.vector.tensor_reduce(tot[:], sums[:],
                                            mybir.AxisListType.X,
                                            mybir.AluOpType.add)
                    rec = wpool.tile([128, 1], F32, tag="rec")
                    nc.vector.reciprocal(rec[:], tot[:])
                    gts = wpool.tile([128, CI], F32, tag="gts")
                    nc.vector.tensor_scalar_mul(
                        gts[:], gt_sb[:, i * CI:(i + 1) * CI], rec[:])

                    gts_prev = gts[:]
                    exp_prev = exp_t

                # last block's y matmuls
                i = NBLK - 1
                for j, (off, cw) in enumerate(Y_CHUNKS):
                    nc.tensor.matmul(
                        y_slot(j)[:, :cw], gts_prev, exp_prev[:, off:off + cw],
                        start=(i == 0), stop=True,
                        tile_position=(0, 32 * (j % 4)),
                        skip_group_check=True)

            # ---- final projection + residual + store ----
            with tc.tile_pool(name="fpsum", bufs=2, space="PSUM") as fpool:
                for j, (off, cw) in enumerate(Y_CHUNKS):
                    p = 32 * (j % 4)
                    ys = y_sb[p:p + 32, (j // 4) * YCH:(j // 4) * YCH + cw]
                    nc.vector.tensor_copy(ys, y_slot(j)[:, :cw])
                    fp = fpool.tile([C, YCH], F32, tag="f")
                    nc.tensor.matmul(fp[:, :cw], wwT4[p:p + 32, :], ys,
                                     start=True, stop=True,
                                     tile_position=(p, 0))
                    ot = wpool.tile([C, YCH], F32, tag="ot")
                    nc.vector.tensor_add(
                        ot[:, :cw], fp[:, :cw], resid[:, off:off + cw])
                    nc.sync.dma_start(out_d[:, off:off + cw], ot[:, :cw])

    nc.compile()
    return nc


def make_in_maps(x, Wg, bg, Wu, bu, Wv, bv, Ww, bw):
    x = np.asarray(x, np.float32)
    ones = np.ones((1, N), np.float32)
    wuT = np.concatenate([np.asarray(Wu, np.float32).T,
                          np.asarray(bu, np.float32)[None, :]], 0)
    wvT = np.concatenate([np.asarray(Wv, np.float32).T,
                          np.asarray(bv, np.float32)[None, :]], 0)
    wgT = np.concatenate([np.asarray(Wg, np.float32).T,
                          np.asarray(bg, np.float32)[None, :]], 0)
    wwT4 = np.concatenate([np.ascontiguousarray(np.asarray(Ww, np.float32).T)] * 4, 0)
    bw = np.asarray(bw, np.float32)

    in_maps = []
    for core in range(NCORES):
        b, h = divmod(core, 2)
        xb = x[b].reshape(C, N)
        x_aug = np.concatenate([xb, ones], 0)
        x_m = np.ascontiguousarray(x_aug[:, h * MH:(h + 1) * MH])
        if h == 1:
            residc = xb + bw[:, None]
        else:
            residc = np.zeros((C, N), np.float32)
        in_maps.append({
            "x_aug": np.ascontiguousarray(x_aug),
            "x_m": x_m,
            "wuT": np.ascontiguousarray(wuT),
            "wvT": np.ascontiguousarray(wvT),
            "wgT": np.ascontiguousarray(wgT),
            "wwT4": np.ascontiguousarray(wwT4),
            "resid": np.ascontiguousarray(residc),
        })
    return in_maps


_NC = None


def kernel(x, Wg, bg, Wu, bu, Wv, bv, Ww, bw, _trace=False):
    global _NC
    if _NC is None:
        _NC = build_nc()
    in_maps = make_in_maps(x, Wg, bg, Wu, bu, Wv, bv, Ww, bw)
    res = run_bass_kernel_spmd(_NC, in_maps, list(range(NCORES)), trace=_trace)
    outs = [r["out"] for r in res.results]
    full = np.empty((B, C, H, W), np.float32)
    for b in range(B):
        full[b] = (outs[2 * b] + outs[2 * b + 1]).reshape(C, H, W)
    kernel.last_results = res
    return full


if __name__ == "__main__":
    rng = np.random.default_rng(0)
    s_in, s_mid = 1.0 / np.sqrt(C), 1.0 / np.sqrt(CI)
    ins = dict(
        x=rng.standard_normal((B, C, H, W), np.float32),
        Wg=(rng.standard_normal((CI, C)) * s_in).astype(np.float32),
        bg=(rng.standard_normal(CI) * 0.01).astype(np.float32),
        Wu=(rng.standard_normal((CI, C)) * s_in).astype(np.float32),
        bu=(rng.standard_normal(CI) * 0.01).astype(np.float32),
        Wv=(rng.standard_normal((CI, C)) * s_in).astype(np.float32),
        bv=(rng.standard_normal(CI) * 0.01).astype(np.float32),
        Ww=(rng.standard_normal((C, CI)) * s_mid).astype(np.float32),
        bw=(rng.standard_normal(C) * 0.01).astype(np.float32),
    )
    out = kernel(**ins)
    print("kernel output", out.shape, out.dtype)


# revision 9
# speedup vs baseline: 2.0168x; 2.0168x over previous
# Trainium2 Bass kernel for NonLocalBlock (B=4, C=64, CI=32, H=W=80).
#
# Math (per batch, N = H*W = 6400):
#   u = Wu@x+bu, v = Wv@x+bv, g = Wg@x+bg           [CI, N]
#   f[n,m] = sum_c u[c,n] v[c,m]; softmax over n (axis=1 of f)
#   y[c,n] = sum_m f_sm[n,m] g[c,m];  out = Ww@y + bw + x
#
# Define S = v^T u  (S[m,n] = f[n,m]).  The softmax axis n is then the
# FREE axis of S rows, so processing S in 128-row blocks makes the
# softmax fully row-local.  y = g @ softmax_rows(S).
#
# Sharding: 8 cores = 4 batches x 2 halves of the m axis.  Each core
# computes a partial y (sum over its 3200 m rows), applies the output
# projection, and the host adds the two halves (bias+residual are
# carried by the odd core via the `resid` input; even cores get zeros).
#
# Numerics: softmax computed WITHOUT max-subtraction: |S| <~ 40 here
# (inputs are unit-normal-ish), exp stays comfortably inside f32 range,
# and exp(S)/sum(exp(S)) is mathematically identical to the reference.
# Row sums come for free from the activation's accum_out; the 1/rowsum
# is folded into the [128,32] g^T block (lhsT of the y matmul) instead
# of rescaling the [128,6400] exp(S) tile.

import numpy as np

import concourse.bass as bass
import concourse.mybir as mybir
from concourse import bacc, tile
from concourse.bass_utils import run_bass_kernel_spmd

F32 = mybir.dt.float32
F32R = mybir.dt.float32r
BF16 = mybir.dt.bfloat16

B, C, CI, H, W = 4, 64, 32, 80, 80
N = H * W              # 6400
NCORES = 8
MH = N // 2            # 3200 rows of S per core
MB = 128               # S row-block
NBLK = MH // MB        # 25 blocks per core
SCH = 1024             # S free-dim chunk held in PSUM (2 banks)
YCH = 512              # y free-dim chunk (1 bank)

EXP = mybir.ActivationFunctionType.Exp


def _ceil_chunks(total, step):
    out = []
    off = 0
    while off < total:
        out.append((off, min(step, total - off)))
        off += step
    return out


S_CHUNKS = _ceil_chunks(N, SCH)      # 6 x 1024 + 256
Y_CHUNKS = _ceil_chunks(N, YCH)      # 12 x 512 + 256
U_CHUNKS = _ceil_chunks(N, 512)      # projection chunks
V_CHUNKS = _ceil_chunks(MH, 512)


def build_nc():
    nc = bacc.Bacc("TRN2", target_bir_lowering=False, debug=False,
                   num_devices=NCORES)

    x_aug_d = nc.dram_tensor("x_aug", [C + 1, N], F32R, kind="ExternalInput")
    x_m_d = nc.dram_tensor("x_m", [C + 1, MH], F32R, kind="ExternalInput")
    wuT_d = nc.dram_tensor("wuT", [C + 1, CI], F32R, kind="ExternalInput")
    wvT_d = nc.dram_tensor("wvT", [C + 1, CI], F32R, kind="ExternalInput")
    wgT_d = nc.dram_tensor("wgT", [C + 1, CI], F32R, kind="ExternalInput")
    wwT4_d = nc.dram_tensor("wwT4", [128, C], F32, kind="ExternalInput")
    resid_d = nc.dram_tensor("resid", [C, N], F32, kind="ExternalInput")
    out_d = nc.dram_tensor("out", [C, N], F32, kind="ExternalOutput")

    with tile.TileContext(nc) as tc:
        with (
            tc.tile_pool(name="const", bufs=1) as cpool,
            tc.tile_pool(name="big", bufs=2) as dpool,
            tc.tile_pool(name="small", bufs=3) as wpool,
            tc.tile_pool(name="ypsum", bufs=1, space="PSUM") as ypool,
        ):
            # ---- persistent SBUF tiles ----
            x_aug = cpool.tile([C + 1, N], F32R, tag="xa")
            x_m = cpool.tile([C + 1, MH], F32R, tag="xm")
            u_sb = cpool.tile([CI, N], F32R, tag="u")
            v_sb = cpool.tile([CI, MH], F32R, tag="v")
            gt_sb = cpool.tile([128, NBLK * CI], F32, tag="gt")
            wuT = cpool.tile([C + 1, CI], F32R, tag="wu")
            wvT = cpool.tile([C + 1, CI], F32R, tag="wv")
            wgT = cpool.tile([C + 1, CI], F32R, tag="wg")
            wwT4 = cpool.tile([128, C], F32, tag="ww")
            resid = cpool.tile([C, N], F32, tag="resid")
            y_sb = cpool.tile([128, 4 * YCH], F32, tag="ysb")

            # ---- input DMAs (chunked for queue parallelism) ----
            for k in range(4):
                s = slice(k * (N // 4), (k + 1) * (N // 4))
                nc.sync.dma_start(x_aug[:, s], x_aug_d[:, s])
            for k in range(2):
                s = slice(k * (MH // 2), (k + 1) * (MH // 2))
                nc.sync.dma_start(x_m[:, s], x_m_d[:, s])
            nc.sync.dma_start(wuT[:], wuT_d[:])
            nc.sync.dma_start(wvT[:], wvT_d[:])
            nc.sync.dma_start(wgT[:], wgT_d[:])
            nc.sync.dma_start(wwT4[:], wwT4_d[:])
            for k in range(4):
                s = slice(k * (N // 4), (k + 1) * (N // 4))
                nc.sync.dma_start(resid[:, s], resid_d[:, s])

            # ---- projections: u (full), v (this core's m range), g^T ----
            with tc.tile_pool(name="ppsum", bufs=2, space="PSUM") as ppool:
                for off, cw in U_CHUNKS:
                    pu = ppool.tile([CI, 512], F32, tag="pu")
                    nc.tensor.matmul(pu[:, :cw], wuT[:], x_aug[:, off:off + cw],
                                     start=True, stop=True)
                    nc.vector.tensor_copy(u_sb[:, off:off + cw], pu[:, :cw])
                for off, cw in V_CHUNKS:
                    pv = ppool.tile([CI, 512], F32, tag="pu")
                    nc.tensor.matmul(pv[:, :cw], wvT[:], x_m[:, off:off + cw],
                                     start=True, stop=True)
                    nc.vector.tensor_copy(v_sb[:, off:off + cw], pv[:, :cw])
                for i in range(NBLK):
                    pg = ppool.tile([128, CI], F32, tag="pg")
                    nc.tensor.matmul(pg[:], x_m[:, i * MB:(i + 1) * MB], wgT[:],
                                     start=True, stop=True)
                    nc.vector.tensor_copy(gt_sb[:, i * CI:(i + 1) * CI], pg[:])

            # ---- y accumulators: 13 chunks packed 4-per-bank ----
            y_ps = [ypool.tile([128, YCH], F32, tag=f"y{t}", name=f"y{t}")
                    for t in range(4)]

            def y_slot(j):
                return y_ps[j // 4][32 * (j % 4):32 * (j % 4) + 32, :]

            gts_prev = None
            exp_prev = None

            with tc.tile_pool(name="spsum", bufs=2, space="PSUM") as spool:
                for i in range(NBLK):
                    vblk = v_sb[:, i * MB:(i + 1) * MB]
                    exp_t = dpool.tile([128, N], BF16, tag="expS")
                    sums = wpool.tile([128, len(S_CHUNKS)], F32, tag="sums")

                    for ci, (off, cw) in enumerate(S_CHUNKS):
                        sp = spool.tile([128, SCH], F32, tag="s")
                        for s2 in range(0, cw, 512):
                            w2 = min(512, cw - s2)
                            nc.tensor.matmul(
                                sp[:, s2:s2 + w2], vblk,
                                u_sb[:, off + s2:off + s2 + w2],
                                start=True, stop=True)
                        nc.scalar.activation(
                            exp_t[:, off:off + cw], sp[:, :cw], EXP,
                            accum_out=sums[:, ci:ci + 1])

                    # y matmuls for the previous block (keeps ACT busy:
                    # this block's S matmuls were already emitted above)
                    if i > 0:
                        for j, (off, cw) in enumerate(Y_CHUNKS):
                            nc.tensor.matmul(
                                y_slot(j)[:, :cw], gts_prev,
                                exp_prev[:, off:off + cw],
                                start=(i - 1 == 0), stop=(i - 1 == NBLK - 1),
                                tile_position=(0, 32 * (j % 4)),
                                skip_group_check=True)

                    tot = wpool.tile([128, 1], F32, tag="tot")
                    nc.vector.tensor_reduce(tot[:], sums[:],
                                            mybir.AxisListType.X,
                                            mybir.AluOpType.add)
                    rec = wpool.tile([128, 1], F32, tag="rec")
                    nc.vector.reciprocal(rec[:], tot[:])
                    gts = wpool.tile([128, CI], BF16, tag="gts")
                    nc.vector.tensor_scalar_mul(
                        gts[:], gt_sb[:, i * CI:(i + 1) * CI], rec[:])

                    gts_prev = gts[:]
                    exp_prev = exp_t

                # last block's y matmuls
                i = NBLK - 1
                for j, (off, cw) in enumerate(Y_CHUNKS):
                    nc.tensor.matmul(
                        y_slot(j)[:, :cw], gts_prev, exp_prev[:, off:off + cw],
                        start=(i == 0), stop=True,
                        tile_position=(0, 32 * (j % 4)),
                        skip_group_check=True)

            # ---- final projection + residual + store ----
            with tc.tile_pool(name="fpsum", bufs=2, space="PSUM") as fpool:
                for j, (off, cw) in enumerate(Y_CHUNKS):
                    p = 32 * (j % 4)
                    ys = y_sb[p:p + 32, (j // 4) * YCH:(j // 4) * YCH + cw]
                    nc.vector.tensor_copy(ys, y_slot(j)[:, :cw])
                    fp = fpool.tile([C, YCH], F32, tag="f")
                    nc.tensor.matmul(fp[:, :cw], wwT4[p:p + 32, :], ys,
                                     start=True, stop=True,
                                     tile_position=(p, 0))
                    ot = wpool.tile([C, YCH], F32, tag="ot")
                    nc.vector.tensor_add(
                        ot[:, :cw], fp[:, :cw], resid[:, off:off + cw])
                    nc.sync.dma_start(out_d[:, off:off + cw], ot[:, :cw])

    nc.compile()
    return nc


def make_in_maps(x, Wg, bg, Wu, bu, Wv, bv, Ww, bw):
    x = np.asarray(x, np.float32)
    ones = np.ones((1, N), np.float32)
    wuT = np.concatenate([np.asarray(Wu, np.float32).T,
                          np.asarray(bu, np.float32)[None, :]], 0)
    wvT = np.concatenate([np.asarray(Wv, np.float32).T,
                          np.asarray(bv, np.float32)[None, :]], 0)
    wgT = np.concatenate([np.asarray(Wg, np.float32).T,
                          np.asarray(bg, np.float32)[None, :]], 0)
    wwT4 = np.concatenate([np.ascontiguousarray(np.asarray(Ww, np.float32).T)] * 4, 0)
    bw = np.asarray(bw, np.float32)

    in_maps = []
    for core in range(NCORES):
        b, h = divmod(core, 2)
        xb = x[b].reshape(C, N)
        x_aug = np.concatenate([xb, ones], 0)
        x_m = np.ascontiguousarray(x_aug[:, h * MH:(h + 1) * MH])
        if h == 1:
            residc = xb + bw[:, None]
        else:
            residc = np.zeros((C, N), np.float32)
        in_maps.append({
            "x_aug": np.ascontiguousarray(x_aug),
            "x_m": x_m,
            "wuT": np.ascontiguousarray(wuT),
            "wvT": np.ascontiguousarray(wvT),
            "wgT": np.ascontiguousarray(wgT),
            "wwT4": np.ascontiguousarray(wwT4),
            "resid": np.ascontiguousarray(residc),
        })
    return in_maps


_NC = None


def kernel(x, Wg, bg, Wu, bu, Wv, bv, Ww, bw, _trace=False):
    global _NC
    if _NC is None:
        _NC = build_nc()
    in_maps = make_in_maps(x, Wg, bg, Wu, bu, Wv, bv, Ww, bw)
    res = run_bass_kernel_spmd(_NC, in_maps, list(range(NCORES)), trace=_trace)
    outs = [r["out"] for r in res.results]
    full = np.empty((B, C, H, W), np.float32)
    for b in range(B):
        full[b] = (outs[2 * b] + outs[2 * b + 1]).reshape(C, H, W)
    kernel.last_results = res
    return full


if __name__ == "__main__":
    rng = np.random.default_rng(0)
    s_in, s_mid = 1.0 / np.sqrt(C), 1.0 / np.sqrt(CI)
    ins = dict(
        x=rng.standard_normal((B, C, H, W), np.float32),
        Wg=(rng.standard_normal((CI, C)) * s_in).astype(np.float32),
        bg=(rng.standard_normal(CI) * 0.01).astype(np.float32),
        Wu=(rng.standard_normal((CI, C)) * s_in).astype(np.float32),
        bu=(rng.standard_normal(CI) * 0.01).astype(np.float32),
        Wv=(rng.standard_normal((CI, C)) * s_in).astype(np.float32),
        bv=(rng.standard_normal(CI) * 0.01).astype(np.float32),
        Ww=(rng.standard_normal((C, CI)) * s_mid).astype(np.float32),
        bw=(rng.standard_normal(C) * 0.01).astype(np.float32),
    )
    out = kernel(**ins)
    print("kernel output", out.shape, out.dtype)


# revision 10
# speedup vs baseline: 2.3905x; 1.1853x over previous
# Trainium2 Bass kernel for NonLocalBlock (B=4, C=64, CI=32, H=W=80).
#
# Math (per batch, N = H*W = 6400):
#   u = Wu@x+bu, v = Wv@x+bv, g = Wg@x+bg           [CI, N]
#   f[n,m] = sum_c u[c,n] v[c,m]; softmax over n (axis=1 of f)
#   y[c,n] = sum_m f_sm[n,m] g[c,m];  out = Ww@y + bw + x
#
# Define S = v^T u  (S[m,n] = f[n,m]).  The softmax axis n is then the
# FREE axis of S rows, so processing S in 128-row blocks makes the
# softmax fully row-local.  y = g @ softmax_rows(S).
#
# Sharding: 8 cores = 4 batches x 2 halves of the m axis.  Each core
# computes a partial y (sum over its 3200 m rows), applies the output
# projection, and the host adds the two halves (bias+residual are
# carried by the odd core via the `resid` input; even cores get zeros).
#
# Numerics: softmax computed WITHOUT max-subtraction: |S| <~ 40 here
# (inputs are unit-normal-ish), exp stays comfortably inside f32 range,
# and exp(S)/sum(exp(S)) is mathematically identical to the reference.
# Row sums come for free from the activation's accum_out; the 1/rowsum
# is folded into the [128,32] g^T block (lhsT of the y matmul) instead
# of rescaling the [128,6400] exp(S) tile.

import numpy as np

import concourse.bass as bass
import concourse.mybir as mybir
from concourse import bacc, tile
from concourse.bass_utils import run_bass_kernel_spmd

F32 = mybir.dt.float32
F32R = mybir.dt.float32r
BF16 = mybir.dt.bfloat16
F16 = mybir.dt.float16

B, C, CI, H, W = 4, 64, 32, 80, 80
N = H * W              # 6400
NCORES = 8
MH = N // 2            # 3200 rows of S per core
MB = 128               # S row-block
NBLK = MH // MB        # 25 blocks per core
SCH = 1024             # S free-dim chunk held in PSUM (2 banks)
YCH = 512              # y free-dim chunk (1 bank)

EXP = mybir.ActivationFunctionType.Exp


def _ceil_chunks(total, step):
    out = []
    off = 0
    while off < total:
        out.append((off, min(step, total - off)))
        off += step
    return out


S_CHUNKS = _ceil_chunks(N, SCH)      # 6 x 1024 + 256
Y_CHUNKS = _ceil_chunks(N, YCH)      # 12 x 512 + 256
U_CHUNKS = _ceil_chunks(N, 512)      # projection chunks
V_CHUNKS = _ceil_chunks(MH, 512)


def build_nc():
    nc = bacc.Bacc("TRN2", target_bir_lowering=False, debug=False,
                   num_devices=NCORES)

    x_aug_d = nc.dram_tensor("x_aug", [C + 1, N], F32R, kind="ExternalInput")
    x_m_d = nc.dram_tensor("x_m", [C + 1, MH], F32R, kind="ExternalInput")
    wuT_d = nc.dram_tensor("wuT", [C + 1, CI], F32R, kind="ExternalInput")
    wvT_d = nc.dram_tensor("wvT", [C + 1, CI], F32R, kind="ExternalInput")
    wgT_d = nc.dram_tensor("wgT", [C + 1, CI], F32R, kind="ExternalInput")
    wwT4_d = nc.dram_tensor("wwT4", [128, C], F32, kind="ExternalInput")
    resid_d = nc.dram_tensor("resid", [C, N], F32, kind="ExternalInput")
    out_d = nc.dram_tensor("out", [C, N], F32, kind="ExternalOutput")

    with tile.TileContext(nc) as tc:
        with (
            tc.tile_pool(name="const", bufs=1) as cpool,
            tc.tile_pool(name="big", bufs=2) as dpool,
            tc.tile_pool(name="small", bufs=3) as wpool,
            tc.tile_pool(name="ypsum", bufs=1, space="PSUM") as ypool,
        ):
            # ---- persistent SBUF tiles ----
            x_aug = cpool.tile([C + 1, N], F32R, tag="xa")
            x_m = cpool.tile([C + 1, MH], F32R, tag="xm")
            u_sb = cpool.tile([2 * CI, N], F16, tag="u")
            v_sb = cpool.tile([2 * CI, MH], F16, tag="v")
            gt_sb = cpool.tile([128, NBLK * CI], F32, tag="gt")
            wuT = cpool.tile([C + 1, CI], F32R, tag="wu")
            wvT = cpool.tile([C + 1, CI], F32R, tag="wv")
            wgT = cpool.tile([C + 1, CI], F32R, tag="wg")
            wwT4 = cpool.tile([128, C], F32, tag="ww")
            resid = cpool.tile([C, N], F32, tag="resid")
            y_sb = cpool.tile([128, 4 * YCH], F32, tag="ysb")

            # ---- input DMAs (chunked for queue parallelism) ----
            for k in range(4):
                s = slice(k * (N // 4), (k + 1) * (N // 4))
                nc.sync.dma_start(x_aug[:, s], x_aug_d[:, s])
            for k in range(2):
                s = slice(k * (MH // 2), (k + 1) * (MH // 2))
                nc.sync.dma_start(x_m[:, s], x_m_d[:, s])
            nc.sync.dma_start(wuT[:], wuT_d[:])
            nc.sync.dma_start(wvT[:], wvT_d[:])
            nc.sync.dma_start(wgT[:], wgT_d[:])
            nc.sync.dma_start(wwT4[:], wwT4_d[:])
            for k in range(4):
                s = slice(k * (N // 4), (k + 1) * (N // 4))
                nc.sync.dma_start(resid[:, s], resid_d[:, s])

            # ---- projections: u (full), v (this core's m range), g^T ----
            with tc.tile_pool(name="ppsum", bufs=2, space="PSUM") as ppool:
                for off, cw in U_CHUNKS:
                    pu = ppool.tile([CI, 512], F32, tag="pu")
                    nc.tensor.matmul(pu[:, :cw], wuT[:], x_aug[:, off:off + cw],
                                     start=True, stop=True)
                    nc.vector.tensor_copy(u_sb[0:CI, off:off + cw], pu[:, :cw])
                for off, cw in V_CHUNKS:
                    pv = ppool.tile([CI, 512], F32, tag="pu")
                    nc.tensor.matmul(pv[:, :cw], wvT[:], x_m[:, off:off + cw],
                                     start=True, stop=True)
                    nc.vector.tensor_copy(v_sb[0:CI, off:off + cw], pv[:, :cw])
                for i in range(NBLK):
                    pg = ppool.tile([128, CI], F32, tag="pg")
                    nc.tensor.matmul(pg[:], x_m[:, i * MB:(i + 1) * MB], wgT[:],
                                     start=True, stop=True)
                    nc.vector.tensor_copy(gt_sb[:, i * CI:(i + 1) * CI], pg[:])
                nc.sync.dma_start(u_sb[CI:2 * CI, :], u_sb[0:CI, :])
                nc.sync.dma_start(v_sb[CI:2 * CI, :], v_sb[0:CI, :])

            # ---- y accumulators: 13 chunks packed 4-per-bank ----
            y_ps = [ypool.tile([128, YCH], F32, tag=f"y{t}", name=f"y{t}")
                    for t in range(4)]

            def y_slot(j):
                return y_ps[j // 4][32 * (j % 4):32 * (j % 4) + 32, :]

            gts_prev = None
            exp_prev = None

            with tc.tile_pool(name="spsum", bufs=2, space="PSUM") as spool:
                mm_idx = 0
                for i in range(NBLK):
                    exp_t = dpool.tile([128, N], BF16, tag="expS")
                    sums = wpool.tile([128, len(S_CHUNKS)], F32, tag="sums")

                    for ci, (off, cw) in enumerate(S_CHUNKS):
                        sp = spool.tile([128, SCH], F32, tag="s")
                        for s2 in range(0, cw, 512):
                            w2 = min(512, cw - s2)
                            g = CI * (mm_idx % 2)
                            mm_idx += 1
                            nc.tensor.matmul(
                                sp[:, s2:s2 + w2],
                                v_sb[g:g + CI, i * MB:(i + 1) * MB],
                                u_sb[g:g + CI, off + s2:off + s2 + w2],
                                start=True, stop=True)
                        nc.scalar.activation(
                            exp_t[:, off:off + cw], sp[:, :cw], EXP,
                            accum_out=sums[:, ci:ci + 1])

                    # y matmuls for the previous block (keeps ACT busy:
                    # this block's S matmuls were already emitted above)
                    if i > 0:
                        for j, (off, cw) in enumerate(Y_CHUNKS):
                            nc.tensor.matmul(
                                y_slot(j)[:, :cw], gts_prev,
                                exp_prev[:, off:off + cw],
                                start=(i - 1 == 0), stop=(i - 1 == NBLK - 1),
                                tile_position=(0, 32 * (j % 4)),
                                skip_group_check=True)

                    tot = wpool.tile([128, 1], F32, tag="tot")
                    nc.vector.tensor_reduce(tot[:], sums[:],
                                            mybir.AxisListType.X,
                                            mybir.AluOpType.add)
                    rec = wpool.tile([128, 1], F32, tag="rec")
                    nc.vector.reciprocal(rec[:], tot[:])
                    gts = wpool.tile([128, CI], BF16, tag="gts")
                    nc.vector.tensor_scalar_mul(
                        gts[:], gt_sb[:, i * CI:(i + 1) * CI], rec[:])

                    gts_prev = gts[:]
                    exp_prev = exp_t

                # last block's y matmuls
                i = NBLK - 1
                for j, (off, cw) in enumerate(Y_CHUNKS):
                    nc.tensor.matmul(
                        y_slot(j)[:, :cw], gts_prev, exp_prev[:, off:off + cw],
                        start=(i == 0), stop=True,
                        tile_position=(0, 32 * (j % 4)),
                        skip_group_check=True)

            # ---- final projection + residual + store ----
            with tc.tile_pool(name="fpsum", bufs=2, space="PSUM") as fpool:
                for j, (off, cw) in enumerate(Y_CHUNKS):
                    p = 32 * (j % 4)
                    ys = y_sb[p:p + 32, (j // 4) * YCH:(j // 4) * YCH + cw]
                    nc.vector.tensor_copy(ys, y_slot(j)[:, :cw])
                    fp = fpool.tile([C, YCH], F32, tag="f")
                    nc.tensor.matmul(fp[:, :cw], wwT4[p:p + 32, :], ys,
                                     start=True, stop=True,
                                     tile_position=(p, 0))
                    ot = wpool.tile([C, YCH], F32, tag="ot")
                    nc.vector.tensor_add(
                        ot[:, :cw], fp[:, :cw], resid[:, off:off + cw])
                    nc.sync.dma_start(out_d[:, off:off + cw], ot[:, :cw])

    nc.compile()
    return nc


def make_in_maps(x, Wg, bg, Wu, bu, Wv, bv, Ww, bw):
    x = np.asarray(x, np.float32)
    ones = np.ones((1, N), np.float32)
    wuT = np.concatenate([np.asarray(Wu, np.float32).T,
                          np.asarray(bu, np.float32)[None, :]], 0)
    wvT = np.concatenate([np.asarray(Wv, np.float32).T,
                          np.asarray(bv, np.float32)[None, :]], 0)
    wgT = np.concatenate([np.asarray(Wg, np.float32).T,
                          np.asarray(bg, np.float32)[None, :]], 0)
    wwT4 = np.concatenate([np.ascontiguousarray(np.asarray(Ww, np.float32).T)] * 4, 0)
    bw = np.asarray(bw, np.float32)

    in_maps = []
    for core in range(NCORES):
        b, h = divmod(core, 2)
        xb = x[b].reshape(C, N)
        x_aug = np.concatenate([xb, ones], 0)
        x_m = np.ascontiguousarray(x_aug[:, h * MH:(h + 1) * MH])
        if h == 1:
            residc = xb + bw[:, None]
        else:
            residc = np.zeros((C, N), np.float32)
        in_maps.append({
            "x_aug": np.ascontiguousarray(x_aug),
            "x_m": x_m,
            "wuT": np.ascontiguousarray(wuT),
            "wvT": np.ascontiguousarray(wvT),
            "wgT": np.ascontiguousarray(wgT),
            "wwT4": np.ascontiguousarray(wwT4),
            "resid": np.ascontiguousarray(residc),
        })
    return in_maps


_NC = None


def kernel(x, Wg, bg, Wu, bu, Wv, bv, Ww, bw, _trace=False):
    global _NC
    if _NC is None:
        _NC = build_nc()
    in_maps = make_in_maps(x, Wg, bg, Wu, bu, Wv, bv, Ww, bw)
    res = run_bass_kernel_spmd(_NC, in_maps, list(range(NCORES)), trace=_trace)
    outs = [r["out"] for r in res.results]
    full = np.empty((B, C, H, W), np.float32)
    for b in range(B):
        full[b] = (outs[2 * b] + outs[2 * b + 1]).reshape(C, H, W)
    kernel.last_results = res
    return full


if __name__ == "__main__":
    rng = np.random.default_rng(0)
    s_in, s_mid = 1.0 / np.sqrt(C), 1.0 / np.sqrt(CI)
    ins = dict(
        x=rng.standard_normal((B, C, H, W), np.float32),
        Wg=(rng.standard_normal((CI, C)) * s_in).astype(np.float32),
        bg=(rng.standard_normal(CI) * 0.01).astype(np.float32),
        Wu=(rng.standard_normal((CI, C)) * s_in).astype(np.float32),
        bu=(rng.standard_normal(CI) * 0.01).astype(np.float32),
        Wv=(rng.standard_normal((CI, C)) * s_in).astype(np.float32),
        bv=(rng.standard_normal(CI) * 0.01).astype(np.float32),
        Ww=(rng.standard_normal((C, CI)) * s_mid).astype(np.float32),
        bw=(rng.standard_normal(C) * 0.01).astype(np.float32),
    )
    out = kernel(**ins)
    print("kernel output", out.shape, out.dtype)


# revision 11
# speedup vs baseline: 2.6046x; 1.0896x over previous
# Trainium2 Bass kernel for NonLocalBlock (B=4, C=64, CI=32, H=W=80).
#
# Math (per batch, N = H*W = 6400):
#   u = Wu@x+bu, v = Wv@x+bv, g = Wg@x+bg           [CI, N]
#   f[n,m] = sum_c u[c,n] v[c,m]; softmax over n (axis=1 of f)
#   y[c,n] = sum_m f_sm[n,m] g[c,m];  out = Ww@y + bw + x
#
# Define S = v^T u  (S[m,n] = f[n,m]).  The softmax axis n is then the
# FREE axis of S rows, so processing S in 128-row blocks makes the
# softmax fully row-local.  y = g @ softmax_rows(S).
#
# Sharding: 8 cores = 4 batches x 2 halves of the m axis.  Each core
# computes a partial y (sum over its 3200 m rows), applies the output
# projection, and the host adds the two halves (bias+residual are
# carried by the odd core via the `resid` input; even cores get zeros).
#
# Numerics: softmax computed WITHOUT max-subtraction: |S| <~ 40 here,
# exp stays inside f32 range, and exp(S)/sum(exp(S)) is mathematically
# identical to the reference.  Row sums come free from the activation's
# accum_out; 1/rowsum is folded into the small [128,32] g^T operand.
# fp16 is used for matmul operands whose range allows it (x, u, v, y —
# all O(10)); exp(S) is stored bf16 (needs the range).  Per-core error
# vs the f64 reference lands ~1.5e-3.
#
# Engine budget per core (target): ACT ~190us (164M exps / 8 cores at
# 1 elem/lane/cycle @1.2GHz + per-instr overhead) is the bottleneck;
# PE ~170us (S and y matmuls at 1 cycle/row via fp16/bf16, LDWEIGHTS
# hidden by alternating PE row groups for S); DVE/DMA far below.

import numpy as np

import concourse.bass as bass
import concourse.mybir as mybir
from concourse import bacc, tile
from concourse.bass_utils import run_bass_kernel_spmd

F32 = mybir.dt.float32
F32R = mybir.dt.float32r
BF16 = mybir.dt.bfloat16
F16 = mybir.dt.float16

B, C, CI, H, W = 4, 64, 32, 80, 80
N = H * W              # 6400
NCORES = 8
MH = N // 2            # 3200 rows of S per core
MB = 128               # S row-block
NBLK = MH // MB        # 25 blocks per core
SCH = 1024             # S free-dim chunk held in PSUM (2 banks)
YCH = 512              # y free-dim chunk (1 bank)

EXP = mybir.ActivationFunctionType.Exp


def _ceil_chunks(total, step):
    out = []
    off = 0
    while off < total:
        out.append((off, min(step, total - off)))
        off += step
    return out


S_CHUNKS = _ceil_chunks(N, SCH)      # 6 x 1024 + 256
Y_CHUNKS = _ceil_chunks(N, YCH)      # 12 x 512 + 256
U_CHUNKS = _ceil_chunks(N, 512)
V_CHUNKS = _ceil_chunks(MH, 512)


def build_nc():
    nc = bacc.Bacc("TRN2", target_bir_lowering=False, debug=False,
                   num_devices=NCORES)

    x_aug_d = nc.dram_tensor("x_aug", [C + 1, N], F16, kind="ExternalInput")
    x_m_d = nc.dram_tensor("x_m", [C + 1, MH], F16, kind="ExternalInput")
    wuT_d = nc.dram_tensor("wuT", [C + 1, CI], F16, kind="ExternalInput")
    wvT_d = nc.dram_tensor("wvT", [C + 1, CI], F16, kind="ExternalInput")
    wgT_d = nc.dram_tensor("wgT", [C + 1, CI], F16, kind="ExternalInput")
    wwT4_d = nc.dram_tensor("wwT4", [128, C], F16, kind="ExternalInput")
    resid_d = nc.dram_tensor("resid", [C, N], F32, kind="ExternalInput")
    out_d = nc.dram_tensor("out", [C, N], F32, kind="ExternalOutput")

    with tile.TileContext(nc) as tc:
        with (
            tc.tile_pool(name="const", bufs=1) as cpool,
            tc.tile_pool(name="big", bufs=2) as dpool,
            tc.tile_pool(name="small", bufs=3) as wpool,
            tc.tile_pool(name="ypsum", bufs=1, space="PSUM") as ypool,
        ):
            # ---- persistent SBUF tiles ----
            x_aug = cpool.tile([C + 1, N], F16, tag="xa")
            x_m = cpool.tile([C + 1, MH], F16, tag="xm")
            u_sb = cpool.tile([2 * CI, N], F16, tag="u")     # 2 row groups
            v_sb = cpool.tile([2 * CI, MH], F16, tag="v")
            gt_sb = cpool.tile([128, NBLK * CI], F32, tag="gt")
            wuT = cpool.tile([C + 1, CI], F16, tag="wu")
            wvT = cpool.tile([C + 1, CI], F16, tag="wv")
            wgT = cpool.tile([C + 1, CI], F16, tag="wg")
            wwT4 = cpool.tile([128, C], F16, tag="ww")
            resid = cpool.tile([C, N], F32, tag="resid")
            y_sb = cpool.tile([128, 4 * YCH], F16, tag="ysb")

            # ---- input DMAs needed for the prologue ----
            nc.sync.dma_start(wuT[:], wuT_d[:])
            nc.sync.dma_start(wvT[:], wvT_d[:])
            nc.sync.dma_start(wgT[:], wgT_d[:])
            for k in range(8):
                s = slice(k * (N // 8), (k + 1) * (N // 8))
                nc.sync.dma_start(x_aug[:, s], x_aug_d[:, s])
            for k in range(4):
                s = slice(k * (MH // 4), (k + 1) * (MH // 4))
                nc.sync.dma_start(x_m[:, s], x_m_d[:, s])

            # ---- projections: u (full), v (this core's m range), g^T ----
            with tc.tile_pool(name="ppsum", bufs=2, space="PSUM") as ppool:
                for off, cw in U_CHUNKS:
                    pu = ppool.tile([CI, 512], F32, tag="pu")
                    nc.tensor.matmul(pu[:, :cw], wuT[:], x_aug[:, off:off + cw],
                                     start=True, stop=True)
                    # ACT does the u copies (DVE is busy with v/g^T)
                    nc.scalar.copy(u_sb[0:CI, off:off + cw], pu[:, :cw])
                    nc.sync.dma_start(u_sb[CI:2 * CI, off:off + cw],
                                      u_sb[0:CI, off:off + cw])
                for off, cw in V_CHUNKS:
                    pv = ppool.tile([CI, 512], F32, tag="pu")
                    nc.tensor.matmul(pv[:, :cw], wvT[:], x_m[:, off:off + cw],
                                     start=True, stop=True)
                    nc.vector.tensor_copy(v_sb[0:CI, off:off + cw], pv[:, :cw])
                    nc.sync.dma_start(v_sb[CI:2 * CI, off:off + cw],
                                      v_sb[0:CI, off:off + cw])
                for i in range(NBLK):
                    pg = ppool.tile([128, CI], F32, tag="pg")
                    nc.tensor.matmul(pg[:], x_m[:, i * MB:(i + 1) * MB], wgT[:],
                                     start=True, stop=True)
                    nc.vector.tensor_copy(gt_sb[:, i * CI:(i + 1) * CI], pg[:])

            # ---- y accumulators: 13 chunks packed 4-per-bank ----
            y_ps = [ypool.tile([128, YCH], F32, tag=f"y{t}", name=f"y{t}")
                    for t in range(4)]

            def y_slot(j):
                return y_ps[j // 4][32 * (j % 4):32 * (j % 4) + 32, :]

            gts_prev = None
            exp_prev = None

            with tc.tile_pool(name="spsum", bufs=2, space="PSUM") as spool:
                mm_idx = 0
                for i in range(NBLK):
                    exp_t = dpool.tile([128, N], BF16, tag="expS")
                    sums = wpool.tile([128, len(S_CHUNKS)], F32, tag="sums")

                    for ci, (off, cw) in enumerate(S_CHUNKS):
                        sp = spool.tile([128, SCH], F32, tag="s")
                        for s2 in range(0, cw, 512):
                            w2 = min(512, cw - s2)
                            g = CI * (mm_idx % 2)   # alternate PE row groups
                            mm_idx += 1
                            nc.tensor.matmul(
                                sp[:, s2:s2 + w2],
                                v_sb[g:g + CI, i * MB:(i + 1) * MB],
                                u_sb[g:g + CI, off + s2:off + s2 + w2],
                                start=True, stop=True)
                        nc.scalar.activation(
                            exp_t[:, off:off + cw], sp[:, :cw], EXP,
                            accum_out=sums[:, ci:ci + 1])

                    # y matmuls for the previous block (emitted after this
                    # block's S matmuls so ACT never starves)
                    if i > 0:
                        for j, (off, cw) in enumerate(Y_CHUNKS):
                            nc.tensor.matmul(
                                y_slot(j)[:, :cw], gts_prev,
                                exp_prev[:, off:off + cw],
                                start=(i - 1 == 0), stop=(i - 1 == NBLK - 1),
                                tile_position=(0, 32 * (j % 4)),
                                skip_group_check=True)

                    tot = wpool.tile([128, 1], F32, tag="tot")
                    nc.vector.tensor_reduce(tot[:], sums[:],
                                            mybir.AxisListType.X,
                                            mybir.AluOpType.add)
                    rec = wpool.tile([128, 1], F32, tag="rec")
                    nc.vector.reciprocal(rec[:], tot[:])
                    gts = wpool.tile([128, CI], BF16, tag="gts")
                    nc.vector.tensor_scalar_mul(
                        gts[:], gt_sb[:, i * CI:(i + 1) * CI], rec[:])

                    gts_prev = gts[:]
                    exp_prev = exp_t

                # residual arrives while the main loop runs
                for k in range(4):
                    s = slice(k * (N // 4), (k + 1) * (N // 4))
                    nc.sync.dma_start(resid[:, s], resid_d[:, s])
                nc.sync.dma_start(wwT4[:], wwT4_d[:])

                # last block's y matmuls
                i = NBLK - 1
                for j, (off, cw) in enumerate(Y_CHUNKS):
                    nc.tensor.matmul(
                        y_slot(j)[:, :cw], gts_prev, exp_prev[:, off:off + cw],
                        start=(i == 0), stop=True,
                        tile_position=(0, 32 * (j % 4)),
                        skip_group_check=True)

            # ---- final projection + residual + store ----
            with tc.tile_pool(name="fpsum", bufs=2, space="PSUM") as fpool:
                for j, (off, cw) in enumerate(Y_CHUNKS):
                    p = 32 * (j % 4)
                    ys = y_sb[p:p + 32, (j // 4) * YCH:(j // 4) * YCH + cw]
                    # ACT does the y copies (fp32 psum -> fp16), DVE the adds
                    nc.scalar.copy(ys, y_slot(j)[:, :cw])
                    fp = fpool.tile([C, YCH], F32, tag="f")
                    nc.tensor.matmul(fp[:, :cw], wwT4[p:p + 32, :], ys,
                                     start=True, stop=True,
                                     tile_position=(p, 0))
                    ot = wpool.tile([C, YCH], F32, tag="ot")
                    nc.vector.tensor_add(
                        ot[:, :cw], fp[:, :cw], resid[:, off:off + cw])
                    nc.sync.dma_start(out_d[:, off:off + cw], ot[:, :cw])

    nc.compile()
    return nc


def make_in_maps(x, Wg, bg, Wu, bu, Wv, bv, Ww, bw):
    x = np.asarray(x, np.float32)
    x16 = x.astype(np.float16)
    ones = np.ones((1, N), np.float16)
    wuT = np.concatenate([np.asarray(Wu, np.float32).T,
                          np.asarray(bu, np.float32)[None, :]], 0).astype(np.float16)
    wvT = np.concatenate([np.asarray(Wv, np.float32).T,
                          np.asarray(bv, np.float32)[None, :]], 0).astype(np.float16)
    wgT = np.concatenate([np.asarray(Wg, np.float32).T,
                          np.asarray(bg, np.float32)[None, :]], 0).astype(np.float16)
    wwT4 = np.concatenate(
        [np.ascontiguousarray(np.asarray(Ww, np.float32).T)] * 4, 0).astype(np.float16)
    bw = np.asarray(bw, np.float32)

    in_maps = []
    for core in range(NCORES):
        b, h = divmod(core, 2)
        xb16 = x16[b].reshape(C, N)
        x_aug = np.concatenate([xb16, ones], 0)
        x_m = np.ascontiguousarray(x_aug[:, h * MH:(h + 1) * MH])
        if h == 1:
            residc = x[b].reshape(C, N) + bw[:, None]
        else:
            residc = np.zeros((C, N), np.float32)
        in_maps.append({
            "x_aug": np.ascontiguousarray(x_aug),
            "x_m": x_m,
            "wuT": np.ascontiguousarray(wuT),
            "wvT": np.ascontiguousarray(wvT),
            "wgT": np.ascontiguousarray(wgT),
            "wwT4": np.ascontiguousarray(wwT4),
            "resid": np.ascontiguousarray(residc),
        })
    return in_maps


_NC = None


def kernel(x, Wg, bg, Wu, bu, Wv, bv, Ww, bw, _trace=False):
    global _NC
    if _NC is None:
        _NC = build_nc()
    in_maps = make_in_maps(x, Wg, bg, Wu, bu, Wv, bv, Ww, bw)
    res = run_bass_kernel_spmd(_NC, in_maps, list(range(NCORES)), trace=_trace)
    outs = [r["out"] for r in res.results]
    full = np.empty((B, C, H, W), np.float32)
    for b in range(B):
        full[b] = (outs[2 * b] + outs[2 * b + 1]).reshape(C, H, W)
    kernel.last_results = res
    return full


if __name__ == "__main__":
    rng = np.random.default_rng(0)
    s_in, s_mid = 1.0 / np.sqrt(C), 1.0 / np.sqrt(CI)
    ins = dict(
        x=rng.standard_normal((B, C, H, W), np.float32),
        Wg=(rng.standard_normal((CI, C)) * s_in).astype(np.float32),
        bg=(rng.standard_normal(CI) * 0.01).astype(np.float32),
        Wu=(rng.standard_normal((CI, C)) * s_in).astype(np.float32),
        bu=(rng.standard_normal(CI) * 0.01).astype(np.float32),
        Wv=(rng.standard_normal((CI, C)) * s_in).astype(np.float32),
        bv=(rng.standard_normal(CI) * 0.01).astype(np.float32),
        Ww=(rng.standard_normal((C, CI)) * s_mid).astype(np.float32),
        bw=(rng.standard_normal(C) * 0.01).astype(np.float32),
    )
    out = kernel(**ins)
    print("kernel output", out.shape, out.dtype)
